# revision 25
# baseline (speedup 1.0000x reference)
"""Multi-head attention (B=2, L=2048, DIM=1024, 16 heads) on 8 trn2 cores.

Sharding: core = (batch b in 0..1) x (head-group hg in 0..3); each core
computes 4 heads of one batch element end-to-end (QKV proj, scores,
softmax, PV, partial out-proj). Host sums the 4 partial projections per
batch element and adds the bias.

v2 schedule (vs baseline):
  - heads processed as 2 PAIRS; the two heads of a pair occupy SBUF
    partition halves 0-63 / 64-127, so their K=64 score matmuls run
    CONCURRENTLY on disjoint PE row-groups (tile_position (0,0)/(64,0))
  - exp issued as one N=2048 ACTIVATE per 2 j-chunks (psS spans 4 PSUM
    banks) to amortize the ~352-cycle ACT instruction overhead
  - normalize: denominator rows staged to SBUF immediately (psO freed
    fast), one batched DVE reciprocal [2,512] per (pair, ic), one K=2
    matmul broadcasts both heads' 1/d to 128 partitions
  - pair-1 QKV and out-proj tiles interleaved as PE filler inside the
    ACT-bound attention loop; inputs DMAed in chunks so the first score
    matmul issues early
"""

import ml_dtypes
import numpy as np

import bass_rust
import concourse.bass as bass
import concourse.tile as tile
from concourse import mybir
from concourse.bass_utils import run_bass_kernel_spmd
from concourse.vector_clock import ScopedClock

# ---- problem constants (hardcoded; kernel.py must be self-contained) ----
B = 2
L = 2048
DIM = 1024
NUM_HEADS = 16
HEAD_DIM = 64
SCALE = HEAD_DIM ** -0.5

NCORES = 8
NH = 4             # heads per core
NPAIR = 2          # head pairs per core
C = NH * HEAD_DIM  # 256 head-cols per core
DA = HEAD_DIM + 1  # V augmented with ones column
KC = DIM // 128    # 8 contraction chunks for qkv proj
JC = L // 128      # 16 key-position chunks
NIC = 4            # query chunks of 512
ICW = 512          # query chunk width

F32 = mybir.dt.float32
F32R = mybir.dt.float32r
BF16 = mybir.dt.bfloat16

# walrus in this container rejects >4 sync waits on one CTRL (drain)
# instruction; split the final TileContext drain into multiple drains.
_MAX_DRAIN_WAITS = 1


def _wait_limit(inst):
    # walrus struct wait-slot capacity varies by opcode; matmul (S3_LW)
    # and DMA structs only fit one sync wait. Use 1 everywhere for safety.
    return 1


def _merge_waits(base, extra):
    """Merge sem waits; same-sem waits collapse to the max wait value."""
    out = {w.id: w for w in base}
    for w in extra:
        cur = out.get(w.id)
        if cur is None or w.wait_value > cur.wait_value:
            out[w.id] = w
    return list(out.values())


def _fix_excess_waits(nc):
    """Walrus encodes at most 1 sync wait per instruction in this build.
    For instructions carrying more, insert ENGINE_NOP wait-carriers
    immediately before them on the same engine stream — semantically
    identical (waits execute at the same stream position)."""
    def make_nop(like_inst):
        eng = nc.engines[like_inst.engine]
        bi = eng.nop(nofuse=True, hint="waitsplit")
        nop = bi.ins if hasattr(bi, "ins") else bi
        # isa() appended it to the current (last) block; pull it out.
        for bb2 in nc.main_func.blocks:
            lst = bb2.instructions
            if lst and lst[-1] is nop:
                lst.pop()
                break
        return nop

    for bb in nc.main_func.blocks:
        insts = bb.instructions  # live list
        i = 0
        while i < len(insts):
            inst = insts[i]
            si = inst.sync_info
            lim = _wait_limit(inst)
            if si is None or not si.on_wait or len(si.on_wait) <= lim:
                i += 1
                continue
            waits = _merge_waits(list(si.on_wait), [])
            if len(waits) <= lim:
                inst.sync_info = bass_rust.SyncInfo(
                    on_wait=waits, on_update=list(si.on_update)
                )
                i += 1
                continue
            keep = waits[-lim:]
            overflow = waits[:-lim]
            for w in overflow:
                nop = make_nop(inst)
                nop.sync_info = bass_rust.SyncInfo(on_wait=[w], on_update=[])
                insts.insert(i, nop)
                i += 1
            inst.sync_info = bass_rust.SyncInfo(
                on_wait=keep, on_update=list(si.on_update)
            )
            i += 1


def _split_drain_and_barrier(self, tick_clock, wait_clock):
    _fix_excess_waits(self.nc)
    drain_inst = self.nc.sync.drain()
    wait_clock.add_sem_waits(
        drain_inst.ins, ScopedClock({None: tick_clock.global_clock})
    )
    si = drain_inst.ins.sync_info
    waits = list(si.on_wait) if si is not None and si.on_wait else []
    if len(waits) > _MAX_DRAIN_WAITS:
        drain_inst.ins.sync_info = bass_rust.SyncInfo(
            on_wait=waits[:_MAX_DRAIN_WAITS], on_update=list(si.on_update)
        )
        rest = waits[_MAX_DRAIN_WAITS:]
        while rest:
            d2 = self.nc.sync.drain()
            d2.ins.sync_info = bass_rust.SyncInfo(
                on_wait=rest[:_MAX_DRAIN_WAITS], on_update=[]
            )
            rest = rest[_MAX_DRAIN_WAITS:]
    self.nc.all_engine_barrier()
    assert self.sems is not None
    popped = self.nc._tile_sem_poison_stack.pop()
    assert popped is self._sem_poison
    # RANGE_CLEAR's count field can't encode large ranges; clear in chunks.
    sems = list(self.sems.allocated().values())
    for k in range(0, len(sems), 16):
        self.nc.clear_and_free_semaphores(sems[k:k + 16])
    self.nc.all_engine_barrier()


tile.TileContext._drain_and_barrier = _split_drain_and_barrier

# This walrus build allows at most 2 sync waits per instruction. Collapse
# all HWDGE DMA completions onto a single semaphore lane so consumers that
# wait on two different DMAed tiles (plus a slot release) stay within the
# limit.
import concourse.tile_sem_assignment as _tsa  # noqa: E402

_tsa.NUM_HWDGE_SEMS = 8



def _build_nc() -> bass.Bass:
    nc = bass.Bass("TRN2", target_bir_lowering=False, debug=False)

    # host-swizzled so each DMA chunk is contiguous per partition:
    # x4[p, lc, kc, l'] = x^T[kc*128+p, lc*512+l']
    x4_h = nc.dram_tensor("x4", [128, 4, KC, 512], BF16, kind="ExternalInput")
    # w6[p, seg, kc, c]; seg = pair*3 + {0:K, 1:Q, 2:V}
    w6_h = nc.dram_tensor("w6", [128, 6, KC, 128], BF16, kind="ExternalInput")
    # wp2[p, cc, o] = w_proj^T[cc*128+p, o]
    wp2_h = nc.dram_tensor("wp2", [128, 2, DIM], BF16, kind="ExternalInput")
    ones_h = nc.dram_tensor("ones", [128, 32], BF16, kind="ExternalInput")
    onesr_h = nc.dram_tensor("onesr", [1, 64], F32, kind="ExternalInput")
    out_h = nc.dram_tensor("out", [L, DIM], BF16, kind="ExternalOutput")

    with tile.TileContext(nc) as tc:
        import contextlib
        with contextlib.ExitStack() as ctx:
            singles = ctx.enter_context(tc.tile_pool(name="singles", bufs=1))
            e_pool = ctx.enter_context(tc.tile_pool(name="e", bufs=3))
            dpool = ctx.enter_context(tc.tile_pool(name="dp", bufs=2))
            vstage = ctx.enter_context(tc.tile_pool(name="vs", bufs=4))
            out_pool = ctx.enter_context(tc.tile_pool(name="outp", bufs=4))

            wT_sb = singles.tile([128, 6, KC, 128], BF16)
            wp_sb = singles.tile([128, 2, DIM], BF16)
            ones_sb = singles.tile([128, 32], BF16)
            onesr_sb = singles.tile([1, 64], F32)
            # x^T chunks: one tile per l-chunk of 512
            xt = [singles.tile([128, KC, 512], BF16, name=f"xt{lc}",
                               tag=f"xt{lc}") for lc in range(4)]
            # Q^T / K^T per pair: [128 (2 heads x 64 dims), L]
            qt = [singles.tile([128, L], BF16, name=f"qt{p}", tag=f"qt{p}")
                  for p in range(NPAIR)]
            kt = [singles.tile([128, L], BF16, name=f"kt{p}", tag=f"kt{p}")
                  for p in range(NPAIR)]
            # V^T [c, l] (cheap w-stationary matmuls), per pair
            vtt = [singles.tile([128, L], BF16, name=f"vtt{p}",
                                tag=f"vtt{p}") for p in range(NPAIR)]
            # V natural layout + ones col, per pair (xbar transpose DMA)
            vt = [singles.tile([128, JC, 2, DA], BF16, name=f"vt{p}",
                               tag=f"vt{p}") for p in range(NPAIR)]
            # normalized O^T as lhsT for the out-proj; per query chunk
            po = [singles.tile([128, NPAIR, ICW], BF16, name=f"po{i}",
                               tag=f"po{i}") for i in range(NIC)]

            # ---- input DMAs (sync HWDGE ring, FIFO): prefix needs
            # w[K0,Q0,V0] + xt0 first; all chunks contiguous/partition ----
            nc.sync.dma_start(wT_sb[:, 0], w6_h[:, 0])   # K0
            nc.sync.dma_start(xt[0][:], x4_h[:, 0])
            nc.sync.dma_start(ones_sb[:], ones_h[:])
            nc.sync.dma_start(wT_sb[:, 1], w6_h[:, 1])   # Q0
            nc.sync.dma_start(wT_sb[:, 2], w6_h[:, 2])   # V0
            nc.sync.dma_start(xt[1][:], x4_h[:, 1])
            nc.sync.dma_start(xt[2][:], x4_h[:, 2])
            nc.sync.dma_start(xt[3][:], x4_h[:, 3])
            nc.sync.dma_start(onesr_sb[:], onesr_h[:])
            nc.sync.dma_start(wT_sb[:, 3], w6_h[:, 3])
            nc.sync.dma_start(wT_sb[:, 4], w6_h[:, 4])
            nc.sync.dma_start(wT_sb[:, 5], w6_h[:, 5])
            nc.sync.dma_start(wp_sb[:], wp2_h[:])

            # ones columns for V (denominator trick), via cheap DVE copy
            for p in range(NPAIR):
                nc.vector.tensor_copy(
                    vt[p][:, :, :, 64:65],
                    ones_sb[:].rearrange("q (a b) -> q a b", a=JC)[:, :, :, None],
                )

            with tc.tile_pool(name="psS", bufs=2, space="PSUM") as psS_pool, \
                 tc.tile_pool(name="psO", bufs=1, space="PSUM") as psO_pool, \
                 tc.tile_pool(name="scr", bufs=2, space="PSUM") as scr:

                # ---- stage-1 work, split into ~1us steps so interleaved
                # filler never delays the scores->exp chain by much ----
                kq_open = {}

                def kqh(p, lc, which, half):
                    # half 0 opens the accumulation tile; half 1 closes it.
                    # While a tile is open no other scr allocation may be
                    # emitted (ring-order discipline).
                    seg = p * 3 + (1 if which == 0 else 0)
                    key = (p, lc, which)
                    if half == 0:
                        kq_open[key] = scr.tile([128, 512], F32, name="ps",
                                                tag="scr")
                    ps = kq_open[key]
                    for kc in range(half * 4, half * 4 + 4):
                        nc.tensor.matmul(
                            ps[:],
                            wT_sb[:, seg, kc, :],
                            xt[lc][:, kc, :],
                            start=(kc == 0),
                            stop=(kc == KC - 1),
                        )
                    if half == 1:
                        dst = qt[p] if which == 0 else kt[p]
                        nc.vector.tensor_copy(
                            dst[:, lc * 512:(lc + 1) * 512], ps[:]
                        )
                        del kq_open[key]

                def kq_unit(p, lc, which):
                    kqh(p, lc, which, 0)
                    kqh(p, lc, which, 1)

                def vth(p, lc, half):
                    # V^T[c, l-chunk] accumulation halves (like kqh); on
                    # close, xbar-transpose each 128-l chunk into the
                    # natural [l, (hh, d)] layout -- no PE, no DVE.
                    seg = p * 3 + 2
                    key = (p, lc, "v")
                    if half == 0:
                        kq_open[key] = scr.tile([128, 512], F32, name="psV",
                                                tag="scr")
                    ps = kq_open[key]
                    for kc in range(half * 4, half * 4 + 4):
                        nc.tensor.matmul(
                            ps[:],
                            wT_sb[:, seg, kc, :],
                            xt[lc][:, kc, :],
                            start=(kc == 0),
                            stop=(kc == KC - 1),
                        )
                    if half == 1:
                        nc.vector.tensor_copy(
                            vtt[p][:, lc * 512:(lc + 1) * 512], ps[:]
                        )
                        del kq_open[key]
                        for lt in range(4):
                            jc = lc * 4 + lt
                            tr = vstage.tile([128, 128], BF16, name="tr",
                                             tag="tr")
                            nc.sync.dma_start(
                                tr[:],
                                vtt[p][:, jc * 128:(jc + 1) * 128],
                                transpose=True,
                            )
                            nc.vector.tensor_copy(
                                vt[p][:, jc, :, 0:64],
                                tr[:].rearrange("q (hh d) -> q hh d", hh=2),
                            )

                def v_unit(p, lc):
                    vth(p, lc, 0)
                    vth(p, lc, 1)

                def proj_tile(ic, it):
                    ot = out_pool.tile([128, DIM], BF16, tag="outp")
                    for oc in range(2):
                        psP = scr.tile([128, 512], F32, tag="scr")
                        for cc in range(2):
                            nc.tensor.matmul(
                                psP[:],
                                po[ic][:, cc, it * 128:(it + 1) * 128],
                                wp_sb[:, cc, oc * 512:(oc + 1) * 512],
                                start=(cc == 0),
                                stop=(cc == 1),
                            )
                        nc.vector.tensor_copy(
                            ot[:, oc * 512:(oc + 1) * 512], psP[:]
                        )
                    l0 = ic * ICW + it * 128
                    nc.sync.dma_start(out_h[l0:l0 + 128, :], ot[:])

                # ---- prefix: pair-0 lc0 projections ----
                kq_unit(0, 0, 1)   # K0(lc0)
                kq_unit(0, 0, 0)   # Q0(lc0)
                v_unit(0, 0)       # V0(lc0)

                def attention_pair(p, sched=None):
                    sched = sched or {}
                    dyn = {}
                    psO = [None, None]
                    e_tiles = {}

                    def scores_block(ic, jj):
                        # both heads of the pair: concurrent row-tiled MMs
                        psS = psS_pool.tile([128, 2, 512], F32, name="psS",
                                            tag="psS")
                        for hh in range(2):
                            nc.tensor.matmul(
                                psS[:, hh, :],
                                kt[p][hh * 64:(hh + 1) * 64,
                                      jj * 128:(jj + 1) * 128],
                                qt[p][hh * 64:(hh + 1) * 64,
                                      ic * ICW:(ic + 1) * ICW],
                                start=True,
                                stop=True,
                            )
                        e = e_pool.tile([128, 2, 512], BF16, tag="e")
                        nc.scalar.activation(
                            e[:], psS[:],
                            mybir.ActivationFunctionType.Exp,
                            scale=float(SCALE),
                        )
                        e_tiles[(ic, jj)] = e

                    def pv_block(ic, jj):
                        e = e_tiles.pop((ic, jj))
                        for hh in range(2):
                            nc.tensor.matmul(
                                psO[hh][:],
                                vt[p][:, jj, hh, :],
                                e[:, hh, :],
                                start=(jj == 0),
                                stop=(jj == JC - 1),
                            )

                    def normalize_b(ic, d):
                        # broadcast raw denominators across partitions by
                        # DMA (stride-0 source), reciprocal on 128
                        # partitions, scale po. Deferred a block so the
                        # DVE-heavy reciprocal never blocks stage-1 copies
                        # at a chunk boundary.
                        rb = scr.tile([128, 512], F32, name="rb", tag="scr")
                        for hh in range(2):
                            nc.tensor.matmul(
                                rb[hh * 64:(hh + 1) * 64, :],
                                onesr_sb[:],
                                d[0:1, hh * 512:(hh + 1) * 512],
                                start=True,
                                stop=True,
                            )
                        rr = dpool.tile([128, 512], F32, name="rr", tag="r")
                        nc.vector.reciprocal(rr[:], rb[:])
                        nc.vector.tensor_mul(
                            po[ic][:, p, :], po[ic][:, p, :], rr[:]
                        )

                    def normalize_a(ic, bt):
                        # stage denominators + po rows out of PSUM fast,
                        # then free psO for the next chunk
                        d = dpool.tile([1, 1024], F32, name="d", tag="d")
                        for hh in range(2):
                            nc.vector.tensor_copy(
                                d[0:1, hh * 512:(hh + 1) * 512],
                                psO[hh][64:65, :],
                            )
                            nc.vector.tensor_copy(
                                po[ic][hh * 64:(hh + 1) * 64, p, :],
                                psO[hh][0:64, :],
                            )
                        dyn.setdefault(bt + 1, []).append(
                            lambda: normalize_b(ic, d)
                        )
                        if p == 1:
                            for k, it in enumerate(range(4)):
                                dyn.setdefault(bt + 2 + 3 * k, []).append(
                                    lambda ic=ic, it=it: proj_tile(ic, it)
                                )

                    for bt in range(NIC * JC + 1):
                        ic, t = divmod(bt, JC)
                        if bt < NIC * JC:
                            if t == 0:
                                psO[0] = psO_pool.tile(
                                    [DA, 512], F32, name="psOA", tag="psOA")
                                psO[1] = psO_pool.tile(
                                    [DA, 512], F32, name="psOB", tag="psOB")
                            scores_block(ic, t)
                        if bt > 0:
                            pic, pt = divmod(bt - 1, JC)
                            pv_block(pic, pt)
                            if pt == JC - 1:
                                normalize_a(pic, bt)
                        for task in sched.get(bt, ()):
                            task()
                        for task in dyn.pop(bt, ()):
                            task()
                    # drain deferred work emitted past the last block
                    for bt in sorted(dyn):
                        for task in dyn[bt]:
                            task()

                # ---- pair 0 schedule: hard-due steps early (dense, PE
                # saturated during ic0), relaxed singles later, all away
                # from chunk boundaries (16/32/48) ----
                def KQH(p, lc, w, h):
                    return lambda: kqh(p, lc, w, h)

                def VTH(p, lc, half):
                    return lambda: vth(p, lc, half)

                sched0 = {
                    0: [KQH(0, 1, 1, 0)],
                    1: [KQH(0, 1, 1, 1)],
                    2: [VTH(0, 1, 0)],
                    3: [VTH(0, 1, 1)],
                    4: [KQH(0, 2, 1, 0)],
                    5: [KQH(0, 2, 1, 1)],
                    6: [VTH(0, 2, 0)],
                    7: [VTH(0, 2, 1)],
                    8: [KQH(0, 3, 1, 0)],
                    9: [KQH(0, 3, 1, 1)],
                    10: [VTH(0, 3, 0)],
                    11: [VTH(0, 3, 1)],
                    12: [KQH(0, 1, 0, 0)],
                    13: [KQH(0, 1, 0, 1)],
                    18: [KQH(1, 0, 1, 0)],
                    19: [KQH(1, 0, 1, 1)],
                    20: [VTH(1, 0, 0)],
                    21: [VTH(1, 0, 1)],
                    24: [KQH(1, 1, 1, 0)],
                    25: [KQH(1, 1, 1, 1)],
                    26: [KQH(0, 2, 0, 0)],
                    27: [KQH(0, 2, 0, 1)],
                    28: [VTH(1, 1, 0)],
                    29: [VTH(1, 1, 1)],
                    36: [KQH(1, 2, 1, 0)],
                    37: [KQH(1, 2, 1, 1)],
                    38: [VTH(1, 2, 0)],
                    39: [VTH(1, 2, 1)],
                    40: [KQH(1, 3, 1, 0)],
                    41: [KQH(1, 3, 1, 1)],
                    42: [KQH(0, 3, 0, 0)],
                    43: [KQH(0, 3, 0, 1)],
                    50: [VTH(1, 3, 0)],
                    51: [VTH(1, 3, 1)],
                    54: [KQH(1, 0, 0, 0)],
                    55: [KQH(1, 0, 0, 1)],
                }
                attention_pair(0, sched=sched0)

                # ---- pair 1: remaining Q projections mid-chunk; proj
                # tiles are scheduled dynamically after each normalize ----
                sched1 = {
                    4: [KQH(1, 1, 0, 0)],
                    5: [KQH(1, 1, 0, 1)],
                    20: [KQH(1, 2, 0, 0)],
                    21: [KQH(1, 2, 0, 1)],
                    36: [KQH(1, 3, 0, 0)],
                    37: [KQH(1, 3, 0, 1)],
                }
                attention_pair(1, sched=sched1)
    return nc


_NC_CACHE = None


def _get_nc():
    global _NC_CACHE
    if _NC_CACHE is None:
        _NC_CACHE = _build_nc()
    return _NC_CACHE


def kernel(x, w_qkv, w_proj, b_proj, _trace=False):
    x = np.asarray(x, dtype=np.float32)
    w_qkv = np.asarray(w_qkv, dtype=np.float32)
    w_proj = np.asarray(w_proj, dtype=np.float32)
    b_proj = np.asarray(b_proj, dtype=np.float32)

    nc = _get_nc()
    in_maps = []
    for b in range(B):
        xT = x[b].T  # [DIM, L]
        # x4[p, lc, kc, l'] = xT[kc*128+p, lc*512+l']
        x4 = np.ascontiguousarray(
            xT.reshape(KC, 128, 4, 512).transpose(1, 2, 0, 3)
        ).astype(ml_dtypes.bfloat16)
        for hg in range(4):
            s = C * hg
            segs = []
            for p in range(NPAIR):
                o = s + p * 128
                segs.append(w_qkv[DIM + o:DIM + o + 128])      # K pair p
                segs.append(w_qkv[o:o + 128])                  # Q pair p
                segs.append(w_qkv[2 * DIM + o:2 * DIM + o + 128])  # V
            w_cat = np.concatenate(segs, 0)  # [768, 1024]
            # w6[p, seg, kc, c] = w_cat[seg*128+c, kc*128+p]
            w6 = np.ascontiguousarray(
                w_cat.reshape(6, 128, KC, 128).transpose(3, 0, 2, 1)
            ).astype(ml_dtypes.bfloat16)
            # wp2[p, cc, o] = w_proj[o, s+cc*128+p]
            wp2 = np.ascontiguousarray(
                w_proj[:, s:s + C].T.reshape(2, 128, DIM).transpose(1, 0, 2)
            ).astype(ml_dtypes.bfloat16)
            in_maps.append({
                "x4": x4,
                "w6": w6,
                "wp2": wp2,
                "ones": np.ones((128, 32), ml_dtypes.bfloat16),
                "onesr": np.ones((1, 64), np.float32),
            })

    res = run_bass_kernel_spmd(nc, in_maps, list(range(NCORES)), trace=_trace)
    parts = [res.results[i]["out"].astype(np.float32) for i in range(NCORES)]
    out = np.stack([
        parts[0] + parts[1] + parts[2] + parts[3],
        parts[4] + parts[5] + parts[6] + parts[7],
    ]).astype(np.float32) + b_proj[None, None, :].astype(np.float32)
    if _trace:
        return out, res
    return out


# revision 26
# speedup vs baseline: 1.0404x; 1.0404x over previous
"""Multi-head attention (B=2, L=2048, DIM=1024, 16 heads) on 8 trn2 cores.

Sharding: core = (batch b in 0..1) x (head-group hg in 0..3); each core
computes 4 heads of one batch element end-to-end (QKV proj, scores,
softmax, PV, partial out-proj). Host sums the 4 partial projections per
batch element and adds the bias.

v2 schedule (vs baseline):
  - heads processed as 2 PAIRS; the two heads of a pair occupy SBUF
    partition halves 0-63 / 64-127, so their K=64 score matmuls run
    CONCURRENTLY on disjoint PE row-groups (tile_position (0,0)/(64,0))
  - exp issued as one N=2048 ACTIVATE per 2 j-chunks (psS spans 4 PSUM
    banks) to amortize the ~352-cycle ACT instruction overhead
  - normalize: denominator rows staged to SBUF immediately (psO freed
    fast), one batched DVE reciprocal [2,512] per (pair, ic), one K=2
    matmul broadcasts both heads' 1/d to 128 partitions
  - pair-1 QKV and out-proj tiles interleaved as PE filler inside the
    ACT-bound attention loop; inputs DMAed in chunks so the first score
    matmul issues early
"""

import ml_dtypes
import numpy as np

import bass_rust
import concourse.bass as bass
import concourse.tile as tile
from concourse import mybir
from concourse.bass_utils import run_bass_kernel_spmd
from concourse.vector_clock import ScopedClock

# ---- problem constants (hardcoded; kernel.py must be self-contained) ----
B = 2
L = 2048
DIM = 1024
NUM_HEADS = 16
HEAD_DIM = 64
SCALE = HEAD_DIM ** -0.5

NCORES = 8
NH = 4             # heads per core
NPAIR = 2          # head pairs per core
C = NH * HEAD_DIM  # 256 head-cols per core
DA = HEAD_DIM + 1  # V augmented with ones column
KC = DIM // 128    # 8 contraction chunks for qkv proj
JC = L // 128      # 16 key-position chunks
NIC = 4            # query chunks of 512
ICW = 512          # query chunk width

F32 = mybir.dt.float32
F32R = mybir.dt.float32r
BF16 = mybir.dt.bfloat16

# walrus in this container rejects >4 sync waits on one CTRL (drain)
# instruction; split the final TileContext drain into multiple drains.
_MAX_DRAIN_WAITS = 1


def _wait_limit(inst):
    # walrus struct wait-slot capacity varies by opcode; matmul (S3_LW)
    # and DMA structs only fit one sync wait. Use 1 everywhere for safety.
    return 1


def _merge_waits(base, extra):
    """Merge sem waits; same-sem waits collapse to the max wait value."""
    out = {w.id: w for w in base}
    for w in extra:
        cur = out.get(w.id)
        if cur is None or w.wait_value > cur.wait_value:
            out[w.id] = w
    return list(out.values())


def _fix_excess_waits(nc):
    """Walrus encodes at most 1 sync wait per instruction in this build.
    For instructions carrying more, insert ENGINE_NOP wait-carriers
    immediately before them on the same engine stream — semantically
    identical (waits execute at the same stream position)."""
    def make_nop(like_inst):
        eng = nc.engines[like_inst.engine]
        bi = eng.nop(nofuse=True, hint="waitsplit")
        nop = bi.ins if hasattr(bi, "ins") else bi
        # isa() appended it to the current (last) block; pull it out.
        for bb2 in nc.main_func.blocks:
            lst = bb2.instructions
            if lst and lst[-1] is nop:
                lst.pop()
                break
        return nop

    for bb in nc.main_func.blocks:
        insts = bb.instructions  # live list
        i = 0
        while i < len(insts):
            inst = insts[i]
            si = inst.sync_info
            lim = _wait_limit(inst)
            if si is None or not si.on_wait or len(si.on_wait) <= lim:
                i += 1
                continue
            waits = _merge_waits(list(si.on_wait), [])
            if len(waits) <= lim:
                inst.sync_info = bass_rust.SyncInfo(
                    on_wait=waits, on_update=list(si.on_update)
                )
                i += 1
                continue
            keep = waits[-lim:]
            overflow = waits[:-lim]
            for w in overflow:
                nop = make_nop(inst)
                nop.sync_info = bass_rust.SyncInfo(on_wait=[w], on_update=[])
                insts.insert(i, nop)
                i += 1
            inst.sync_info = bass_rust.SyncInfo(
                on_wait=keep, on_update=list(si.on_update)
            )
            i += 1


def _split_drain_and_barrier(self, tick_clock, wait_clock):
    _fix_excess_waits(self.nc)
    drain_inst = self.nc.sync.drain()
    wait_clock.add_sem_waits(
        drain_inst.ins, ScopedClock({None: tick_clock.global_clock})
    )
    si = drain_inst.ins.sync_info
    waits = list(si.on_wait) if si is not None and si.on_wait else []
    if len(waits) > _MAX_DRAIN_WAITS:
        drain_inst.ins.sync_info = bass_rust.SyncInfo(
            on_wait=waits[:_MAX_DRAIN_WAITS], on_update=list(si.on_update)
        )
        rest = waits[_MAX_DRAIN_WAITS:]
        while rest:
            d2 = self.nc.sync.drain()
            d2.ins.sync_info = bass_rust.SyncInfo(
                on_wait=rest[:_MAX_DRAIN_WAITS], on_update=[]
            )
            rest = rest[_MAX_DRAIN_WAITS:]
    self.nc.all_engine_barrier()
    assert self.sems is not None
    popped = self.nc._tile_sem_poison_stack.pop()
    assert popped is self._sem_poison
    # RANGE_CLEAR's count field can't encode large ranges; clear in chunks.
    sems = list(self.sems.allocated().values())
    for k in range(0, len(sems), 16):
        self.nc.clear_and_free_semaphores(sems[k:k + 16])
    self.nc.all_engine_barrier()


tile.TileContext._drain_and_barrier = _split_drain_and_barrier

# This walrus build allows at most 2 sync waits per instruction. Collapse
# all HWDGE DMA completions onto a single semaphore lane so consumers that
# wait on two different DMAed tiles (plus a slot release) stay within the
# limit.
import concourse.tile_sem_assignment as _tsa  # noqa: E402

_tsa.NUM_HWDGE_SEMS = 8



def _build_nc() -> bass.Bass:
    nc = bass.Bass("TRN2", target_bir_lowering=False, debug=False)

    # host-swizzled so each DMA chunk is contiguous per partition:
    # x4[p, lc, kc, l'] = x^T[kc*128+p, lc*512+l']
    x4_h = nc.dram_tensor("x4", [128, 4, KC, 512], BF16, kind="ExternalInput")
    # w6[p, seg, kc, c]; seg = pair*3 + {0:K, 1:Q, 2:V}
    w6_h = nc.dram_tensor("w6", [128, 6, KC, 128], BF16, kind="ExternalInput")
    # wp2[p, cc, o] = w_proj^T[cc*128+p, o]
    wp2_h = nc.dram_tensor("wp2", [128, 2, DIM], BF16, kind="ExternalInput")
    ones_h = nc.dram_tensor("ones", [128, 32], BF16, kind="ExternalInput")
    onesr_h = nc.dram_tensor("onesr", [1, 64], F32, kind="ExternalInput")
    out_h = nc.dram_tensor("out", [L, DIM], BF16, kind="ExternalOutput")

    with tile.TileContext(nc) as tc:
        import contextlib
        with contextlib.ExitStack() as ctx:
            singles = ctx.enter_context(tc.tile_pool(name="singles", bufs=1))
            e_pool = ctx.enter_context(tc.tile_pool(name="e", bufs=3))
            dpool = ctx.enter_context(tc.tile_pool(name="dp", bufs=2))
            out_pool = ctx.enter_context(tc.tile_pool(name="outp", bufs=4))

            wT_sb = singles.tile([128, 6, KC, 128], BF16)
            wp_sb = singles.tile([128, 2, DIM], BF16)
            ones_sb = singles.tile([128, 32], BF16)
            onesr_sb = singles.tile([1, 64], F32)
            # x^T chunks: one tile per l-chunk of 512
            xt = [singles.tile([128, KC, 512], BF16, name=f"xt{lc}",
                               tag=f"xt{lc}") for lc in range(4)]
            # Q^T / K^T per pair: [128 (2 heads x 64 dims), L]
            qt = [singles.tile([128, L], BF16, name=f"qt{p}", tag=f"qt{p}")
                  for p in range(NPAIR)]
            kt = [singles.tile([128, L], BF16, name=f"kt{p}", tag=f"kt{p}")
                  for p in range(NPAIR)]
            # V natural layout + ones col, per pair
            vt = [singles.tile([128, JC, 2, DA], BF16, name=f"vt{p}",
                               tag=f"vt{p}") for p in range(NPAIR)]
            # normalized O^T as lhsT for the out-proj; per query chunk
            po = [singles.tile([128, NPAIR, ICW], BF16, name=f"po{i}",
                               tag=f"po{i}") for i in range(NIC)]

            # ---- input DMAs (sync HWDGE ring, FIFO): prefix needs
            # w[K0,Q0,V0] + xt0 first; all chunks contiguous/partition ----
            nc.sync.dma_start(wT_sb[:, 0], w6_h[:, 0])   # K0
            nc.sync.dma_start(xt[0][:], x4_h[:, 0])
            nc.sync.dma_start(ones_sb[:], ones_h[:])
            nc.sync.dma_start(wT_sb[:, 1], w6_h[:, 1])   # Q0
            nc.sync.dma_start(wT_sb[:, 2], w6_h[:, 2])   # V0
            nc.sync.dma_start(xt[1][:], x4_h[:, 1])
            nc.sync.dma_start(xt[2][:], x4_h[:, 2])
            nc.sync.dma_start(xt[3][:], x4_h[:, 3])
            nc.sync.dma_start(onesr_sb[:], onesr_h[:])
            nc.sync.dma_start(wT_sb[:, 3], w6_h[:, 3])
            nc.sync.dma_start(wT_sb[:, 4], w6_h[:, 4])
            nc.sync.dma_start(wT_sb[:, 5], w6_h[:, 5])
            nc.sync.dma_start(wp_sb[:], wp2_h[:])

            # ones columns for V (denominator trick), via cheap DVE copy
            for p in range(NPAIR):
                nc.vector.tensor_copy(
                    vt[p][:, :, :, 64:65],
                    ones_sb[:].rearrange("q (a b) -> q a b", a=JC)[:, :, :, None],
                )

            with tc.tile_pool(name="psS", bufs=2, space="PSUM") as psS_pool, \
                 tc.tile_pool(name="psO", bufs=1, space="PSUM") as psO_pool, \
                 tc.tile_pool(name="scr", bufs=2, space="PSUM") as scr:

                # ---- stage-1 work, split into ~1us steps so interleaved
                # filler never delays the scores->exp chain by much ----
                kq_open = {}

                def kqh(p, lc, which, half):
                    # half 0 opens the accumulation tile; half 1 closes it.
                    # While a tile is open no other scr allocation may be
                    # emitted (ring-order discipline).
                    seg = p * 3 + (1 if which == 0 else 0)
                    key = (p, lc, which)
                    if half == 0:
                        kq_open[key] = scr.tile([128, 512], F32, name="ps",
                                                tag="scr")
                    ps = kq_open[key]
                    for kc in range(half * 4, half * 4 + 4):
                        nc.tensor.matmul(
                            ps[:],
                            wT_sb[:, seg, kc, :],
                            xt[lc][:, kc, :],
                            start=(kc == 0),
                            stop=(kc == KC - 1),
                        )
                    if half == 1:
                        dst = qt[p] if which == 0 else kt[p]
                        nc.vector.tensor_copy(
                            dst[:, lc * 512:(lc + 1) * 512], ps[:]
                        )
                        del kq_open[key]

                def kq_unit(p, lc, which):
                    kqh(p, lc, which, 0)
                    kqh(p, lc, which, 1)

                def vlt(p, lc, lt):
                    seg = p * 3 + 2
                    psv = scr.tile([128, 128], F32, name="psv", tag="scr")
                    for kc in range(KC):
                        nc.tensor.matmul(
                            psv[:],
                            xt[lc][:, kc, lt * 128:(lt + 1) * 128],
                            wT_sb[:, seg, kc, :],
                            start=(kc == 0),
                            stop=(kc == KC - 1),
                        )
                    jc = lc * 4 + lt
                    nc.vector.tensor_copy(
                        vt[p][:, jc, :, 0:64],
                        psv[:].rearrange("q (hh d) -> q hh d", hh=2),
                    )

                def v_unit(p, lc):
                    for lt in range(4):
                        vlt(p, lc, lt)

                def proj_tile(ic, it):
                    ot = out_pool.tile([128, DIM], BF16, tag="outp")
                    for oc in range(2):
                        psP = scr.tile([128, 512], F32, tag="scr")
                        for cc in range(2):
                            nc.tensor.matmul(
                                psP[:],
                                po[ic][:, cc, it * 128:(it + 1) * 128],
                                wp_sb[:, cc, oc * 512:(oc + 1) * 512],
                                start=(cc == 0),
                                stop=(cc == 1),
                            )
                        nc.vector.tensor_copy(
                            ot[:, oc * 512:(oc + 1) * 512], psP[:]
                        )
                    l0 = ic * ICW + it * 128
                    nc.sync.dma_start(out_h[l0:l0 + 128, :], ot[:])

                # ---- prefix: pair-0 lc0 projections ----
                kq_unit(0, 0, 1)   # K0(lc0)
                kq_unit(0, 0, 0)   # Q0(lc0)
                v_unit(0, 0)       # V0(lc0)

                def attention_pair(p, sched=None):
                    sched = sched or {}
                    dyn = {}
                    psO = [None, None]
                    e_tiles = {}

                    def scores_block(ic, jj):
                        # both heads of the pair: concurrent row-tiled MMs
                        psS = psS_pool.tile([128, 2, 512], F32, name="psS",
                                            tag="psS")
                        for hh in range(2):
                            nc.tensor.matmul(
                                psS[:, hh, :],
                                kt[p][hh * 64:(hh + 1) * 64,
                                      jj * 128:(jj + 1) * 128],
                                qt[p][hh * 64:(hh + 1) * 64,
                                      ic * ICW:(ic + 1) * ICW],
                                start=True,
                                stop=True,
                            )
                        e = e_pool.tile([128, 2, 512], BF16, tag="e")
                        nc.scalar.activation(
                            e[:], psS[:],
                            mybir.ActivationFunctionType.Exp,
                            scale=float(SCALE),
                        )
                        e_tiles[(ic, jj)] = e

                    def pv_block(ic, jj):
                        e = e_tiles.pop((ic, jj))
                        for hh in range(2):
                            nc.tensor.matmul(
                                psO[hh][:],
                                vt[p][:, jj, hh, :],
                                e[:, hh, :],
                                start=(jj == 0),
                                stop=(jj == JC - 1),
                            )

                    def normalize_b(ic, d):
                        # broadcast raw denominators across partitions by
                        # DMA (stride-0 source), reciprocal on 128
                        # partitions, scale po. Deferred a block so the
                        # DVE-heavy reciprocal never blocks stage-1 copies
                        # at a chunk boundary.
                        rb = scr.tile([128, 512], F32, name="rb", tag="scr")
                        for hh in range(2):
                            nc.tensor.matmul(
                                rb[hh * 64:(hh + 1) * 64, :],
                                onesr_sb[:],
                                d[0:1, hh * 512:(hh + 1) * 512],
                                start=True,
                                stop=True,
                            )
                        rr = dpool.tile([128, 512], F32, name="rr", tag="r")
                        nc.vector.reciprocal(rr[:], rb[:])
                        nc.vector.tensor_mul(
                            po[ic][:, p, :], po[ic][:, p, :], rr[:]
                        )

                    def normalize_a(ic, bt):
                        # stage denominators + po rows out of PSUM fast,
                        # then free psO for the next chunk
                        d = dpool.tile([1, 1024], F32, name="d", tag="d")
                        for hh in range(2):
                            nc.vector.tensor_copy(
                                d[0:1, hh * 512:(hh + 1) * 512],
                                psO[hh][64:65, :],
                            )
                            nc.vector.tensor_copy(
                                po[ic][hh * 64:(hh + 1) * 64, p, :],
                                psO[hh][0:64, :],
                            )
                        dyn.setdefault(bt + 1, []).append(
                            lambda: normalize_b(ic, d)
                        )
                        if p == 1:
                            for k, it in enumerate(range(4)):
                                dyn.setdefault(bt + 2 + 3 * k, []).append(
                                    lambda ic=ic, it=it: proj_tile(ic, it)
                                )

                    for bt in range(NIC * JC + 1):
                        ic, t = divmod(bt, JC)
                        if bt < NIC * JC:
                            if t == 0:
                                psO[0] = psO_pool.tile(
                                    [DA, 512], F32, name="psOA", tag="psOA")
                                psO[1] = psO_pool.tile(
                                    [DA, 512], F32, name="psOB", tag="psOB")
                            scores_block(ic, t)
                        if bt > 0:
                            pic, pt = divmod(bt - 1, JC)
                            pv_block(pic, pt)
                            if pt == JC - 1:
                                normalize_a(pic, bt)
                        for task in sched.get(bt, ()):
                            task()
                        for task in dyn.pop(bt, ()):
                            task()
                    # drain deferred work emitted past the last block
                    for bt in sorted(dyn):
                        for task in dyn[bt]:
                            task()

                # ---- pair 0 schedule: hard-due steps early (dense, PE
                # saturated during ic0), relaxed singles later, all away
                # from chunk boundaries (16/32/48) ----
                def KQH(p, lc, w, h):
                    return lambda: kqh(p, lc, w, h)

                def VLT(p, lc, lt):
                    return lambda: vlt(p, lc, lt)

                sched0 = {
                    0: [KQH(0, 1, 1, 0)],
                    1: [KQH(0, 1, 1, 1)],
                    2: [VLT(0, 1, 0), VLT(0, 1, 1)],
                    3: [VLT(0, 1, 2), VLT(0, 1, 3)],
                    4: [KQH(0, 2, 1, 0)],
                    5: [KQH(0, 2, 1, 1)],
                    6: [VLT(0, 2, 0), VLT(0, 2, 1)],
                    7: [VLT(0, 2, 2), VLT(0, 2, 3)],
                    8: [KQH(0, 3, 1, 0)],
                    9: [KQH(0, 3, 1, 1)],
                    10: [VLT(0, 3, 0), VLT(0, 3, 1)],
                    11: [VLT(0, 3, 2), VLT(0, 3, 3)],
                    12: [KQH(0, 1, 0, 0)],
                    13: [KQH(0, 1, 0, 1)],
                    18: [KQH(1, 0, 1, 0)],
                    19: [KQH(1, 0, 1, 1)],
                    20: [VLT(1, 0, 0)],
                    21: [VLT(1, 0, 1)],
                    22: [VLT(1, 0, 2)],
                    23: [VLT(1, 0, 3)],
                    24: [KQH(1, 1, 1, 0)],
                    25: [KQH(1, 1, 1, 1)],
                    26: [KQH(0, 2, 0, 0)],
                    27: [KQH(0, 2, 0, 1)],
                    28: [VLT(1, 1, 0)],
                    29: [VLT(1, 1, 1)],
                    34: [VLT(1, 1, 2)],
                    35: [VLT(1, 1, 3)],
                    36: [KQH(1, 2, 1, 0)],
                    37: [KQH(1, 2, 1, 1)],
                    38: [VLT(1, 2, 0)],
                    39: [VLT(1, 2, 1)],
                    40: [KQH(1, 3, 1, 0)],
                    41: [KQH(1, 3, 1, 1)],
                    42: [KQH(0, 3, 0, 0)],
                    43: [KQH(0, 3, 0, 1)],
                    44: [VLT(1, 2, 2)],
                    45: [VLT(1, 2, 3)],
                    50: [VLT(1, 3, 0)],
                    51: [VLT(1, 3, 1)],
                    52: [VLT(1, 3, 2)],
                    53: [VLT(1, 3, 3)],
                    54: [KQH(1, 0, 0, 0)],
                    55: [KQH(1, 0, 0, 1)],
                }
                attention_pair(0, sched=sched0)

                # ---- pair 1: remaining Q projections mid-chunk; proj
                # tiles are scheduled dynamically after each normalize ----
                sched1 = {
                    4: [KQH(1, 1, 0, 0)],
                    5: [KQH(1, 1, 0, 1)],
                    20: [KQH(1, 2, 0, 0)],
                    21: [KQH(1, 2, 0, 1)],
                    36: [KQH(1, 3, 0, 0)],
                    37: [KQH(1, 3, 0, 1)],
                }
                attention_pair(1, sched=sched1)
    return nc


_NC_CACHE = None


def _get_nc():
    global _NC_CACHE
    if _NC_CACHE is None:
        _NC_CACHE = _build_nc()
    return _NC_CACHE


def kernel(x, w_qkv, w_proj, b_proj, _trace=False):
    x = np.asarray(x, dtype=np.float32)
    w_qkv = np.asarray(w_qkv, dtype=np.float32)
    w_proj = np.asarray(w_proj, dtype=np.float32)
    b_proj = np.asarray(b_proj, dtype=np.float32)

    nc = _get_nc()
    in_maps = []
    for b in range(B):
        xT = x[b].T  # [DIM, L]
        # x4[p, lc, kc, l'] = xT[kc*128+p, lc*512+l']
        x4 = np.ascontiguousarray(
            xT.reshape(KC, 128, 4, 512).transpose(1, 2, 0, 3)
        ).astype(ml_dtypes.bfloat16)
        for hg in range(4):
            s = C * hg
            segs = []
            for p in range(NPAIR):
                o = s + p * 128
                segs.append(w_qkv[DIM + o:DIM + o + 128])      # K pair p
                segs.append(w_qkv[o:o + 128])                  # Q pair p
                segs.append(w_qkv[2 * DIM + o:2 * DIM + o + 128])  # V
            w_cat = np.concatenate(segs, 0)  # [768, 1024]
            # w6[p, seg, kc, c] = w_cat[seg*128+c, kc*128+p]
            w6 = np.ascontiguousarray(
                w_cat.reshape(6, 128, KC, 128).transpose(3, 0, 2, 1)
            ).astype(ml_dtypes.bfloat16)
            # wp2[p, cc, o] = w_proj[o, s+cc*128+p]
            wp2 = np.ascontiguousarray(
                w_proj[:, s:s + C].T.reshape(2, 128, DIM).transpose(1, 0, 2)
            ).astype(ml_dtypes.bfloat16)
            in_maps.append({
                "x4": x4,
                "w6": w6,
                "wp2": wp2,
                "ones": np.ones((128, 32), ml_dtypes.bfloat16),
                "onesr": np.ones((1, 64), np.float32),
            })

    res = run_bass_kernel_spmd(nc, in_maps, list(range(NCORES)), trace=_trace)
    parts = [res.results[i]["out"].astype(np.float32) for i in range(NCORES)]
    out = np.stack([
        parts[0] + parts[1] + parts[2] + parts[3],
        parts[4] + parts[5] + parts[6] + parts[7],
    ]).astype(np.float32) + b_proj[None, None, :].astype(np.float32)
    if _trace:
        return out, res
    return out


# revision 27
# speedup vs baseline: 1.0441x; 1.0035x over previous
"""Multi-head attention (B=2, L=2048, DIM=1024, 16 heads) on 8 trn2 cores.

Sharding: core = (batch b in 0..1) x (head-group hg in 0..3); each core
computes 4 heads of one batch element end-to-end (QKV proj, scores,
softmax, PV, partial out-proj). Host sums the 4 partial projections per
batch element and adds the bias.

v2 schedule (vs baseline):
  - heads processed as 2 PAIRS; the two heads of a pair occupy SBUF
    partition halves 0-63 / 64-127, so their K=64 score matmuls run
    CONCURRENTLY on disjoint PE row-groups (tile_position (0,0)/(64,0))
  - exp issued as one N=2048 ACTIVATE per 2 j-chunks (psS spans 4 PSUM
    banks) to amortize the ~352-cycle ACT instruction overhead
  - normalize: denominator rows staged to SBUF immediately (psO freed
    fast), one batched DVE reciprocal [2,512] per (pair, ic), one K=2
    matmul broadcasts both heads' 1/d to 128 partitions
  - pair-1 QKV and out-proj tiles interleaved as PE filler inside the
    ACT-bound attention loop; inputs DMAed in chunks so the first score
    matmul issues early
"""

import ml_dtypes
import numpy as np

import bass_rust
import concourse.bass as bass
import concourse.tile as tile
from concourse import mybir
from concourse.bass_utils import run_bass_kernel_spmd
from concourse.vector_clock import ScopedClock

# ---- problem constants (hardcoded; kernel.py must be self-contained) ----
B = 2
L = 2048
DIM = 1024
NUM_HEADS = 16
HEAD_DIM = 64
SCALE = HEAD_DIM ** -0.5

NCORES = 8
NH = 4             # heads per core
NPAIR = 2          # head pairs per core
C = NH * HEAD_DIM  # 256 head-cols per core
DA = HEAD_DIM + 1  # V augmented with ones column
KC = DIM // 128    # 8 contraction chunks for qkv proj
JC = L // 128      # 16 key-position chunks
NIC = 4            # query chunks of 512
ICW = 512          # query chunk width

F32 = mybir.dt.float32
F32R = mybir.dt.float32r
BF16 = mybir.dt.bfloat16

# walrus in this container rejects >4 sync waits on one CTRL (drain)
# instruction; split the final TileContext drain into multiple drains.
_MAX_DRAIN_WAITS = 1


def _wait_limit(inst):
    # walrus struct wait-slot capacity varies by opcode; matmul (S3_LW)
    # and DMA structs only fit one sync wait. Use 1 everywhere for safety.
    return 1


def _merge_waits(base, extra):
    """Merge sem waits; same-sem waits collapse to the max wait value."""
    out = {w.id: w for w in base}
    for w in extra:
        cur = out.get(w.id)
        if cur is None or w.wait_value > cur.wait_value:
            out[w.id] = w
    return list(out.values())


def _fix_excess_waits(nc):
    """Walrus encodes at most 1 sync wait per instruction in this build.
    For instructions carrying more, insert ENGINE_NOP wait-carriers
    immediately before them on the same engine stream — semantically
    identical (waits execute at the same stream position)."""
    def make_nop(like_inst):
        eng = nc.engines[like_inst.engine]
        bi = eng.nop(nofuse=True, hint="waitsplit")
        nop = bi.ins if hasattr(bi, "ins") else bi
        # isa() appended it to the current (last) block; pull it out.
        for bb2 in nc.main_func.blocks:
            lst = bb2.instructions
            if lst and lst[-1] is nop:
                lst.pop()
                break
        return nop

    for bb in nc.main_func.blocks:
        insts = bb.instructions  # live list
        i = 0
        while i < len(insts):
            inst = insts[i]
            si = inst.sync_info
            lim = _wait_limit(inst)
            if si is None or not si.on_wait or len(si.on_wait) <= lim:
                i += 1
                continue
            waits = _merge_waits(list(si.on_wait), [])
            if len(waits) <= lim:
                inst.sync_info = bass_rust.SyncInfo(
                    on_wait=waits, on_update=list(si.on_update)
                )
                i += 1
                continue
            keep = waits[-lim:]
            overflow = waits[:-lim]
            for w in overflow:
                nop = make_nop(inst)
                nop.sync_info = bass_rust.SyncInfo(on_wait=[w], on_update=[])
                insts.insert(i, nop)
                i += 1
            inst.sync_info = bass_rust.SyncInfo(
                on_wait=keep, on_update=list(si.on_update)
            )
            i += 1


def _split_drain_and_barrier(self, tick_clock, wait_clock):
    _fix_excess_waits(self.nc)
    drain_inst = self.nc.sync.drain()
    wait_clock.add_sem_waits(
        drain_inst.ins, ScopedClock({None: tick_clock.global_clock})
    )
    si = drain_inst.ins.sync_info
    waits = list(si.on_wait) if si is not None and si.on_wait else []
    if len(waits) > _MAX_DRAIN_WAITS:
        drain_inst.ins.sync_info = bass_rust.SyncInfo(
            on_wait=waits[:_MAX_DRAIN_WAITS], on_update=list(si.on_update)
        )
        rest = waits[_MAX_DRAIN_WAITS:]
        while rest:
            d2 = self.nc.sync.drain()
            d2.ins.sync_info = bass_rust.SyncInfo(
                on_wait=rest[:_MAX_DRAIN_WAITS], on_update=[]
            )
            rest = rest[_MAX_DRAIN_WAITS:]
    self.nc.all_engine_barrier()
    assert self.sems is not None
    popped = self.nc._tile_sem_poison_stack.pop()
    assert popped is self._sem_poison
    # RANGE_CLEAR's count field can't encode large ranges; clear in chunks.
    sems = list(self.sems.allocated().values())
    for k in range(0, len(sems), 16):
        self.nc.clear_and_free_semaphores(sems[k:k + 16])
    self.nc.all_engine_barrier()


tile.TileContext._drain_and_barrier = _split_drain_and_barrier

# This walrus build allows at most 2 sync waits per instruction. Collapse
# all HWDGE DMA completions onto a single semaphore lane so consumers that
# wait on two different DMAed tiles (plus a slot release) stay within the
# limit.
import concourse.tile_sem_assignment as _tsa  # noqa: E402

_tsa.NUM_HWDGE_SEMS = 8



def _build_nc() -> bass.Bass:
    nc = bass.Bass("TRN2", target_bir_lowering=False, debug=False)

    # host-swizzled so each DMA chunk is contiguous per partition:
    # x4[p, lc, kc, l'] = x^T[kc*128+p, lc*512+l']
    x4_h = nc.dram_tensor("x4", [128, 4, KC, 512], BF16, kind="ExternalInput")
    # w6[p, seg, kc, c]; seg = pair*3 + {0:K, 1:Q, 2:V}
    w6_h = nc.dram_tensor("w6", [128, 6, KC, 128], BF16, kind="ExternalInput")
    # wp2[p, cc, o] = w_proj^T[cc*128+p, o]
    wp2_h = nc.dram_tensor("wp2", [128, 2, DIM], BF16, kind="ExternalInput")
    ones_h = nc.dram_tensor("ones", [128, 32], BF16, kind="ExternalInput")
    onesr_h = nc.dram_tensor("onesr", [1, 64], F32, kind="ExternalInput")
    out_h = nc.dram_tensor("out", [L, DIM], BF16, kind="ExternalOutput")

    with tile.TileContext(nc) as tc:
        import contextlib
        with contextlib.ExitStack() as ctx:
            singles = ctx.enter_context(tc.tile_pool(name="singles", bufs=1))
            e_pool = ctx.enter_context(tc.tile_pool(name="e", bufs=4))
            dpool = ctx.enter_context(tc.tile_pool(name="dp", bufs=3))
            out_pool = ctx.enter_context(tc.tile_pool(name="outp", bufs=6))

            wT_sb = singles.tile([128, 6, KC, 128], BF16)
            wp_sb = singles.tile([128, 2, DIM], BF16)
            ones_sb = singles.tile([128, 32], BF16)
            onesr_sb = singles.tile([1, 64], F32)
            # x^T chunks: one tile per l-chunk of 512
            xt = [singles.tile([128, KC, 512], BF16, name=f"xt{lc}",
                               tag=f"xt{lc}") for lc in range(4)]
            # Q^T / K^T per pair: [128 (2 heads x 64 dims), L]
            qt = [singles.tile([128, L], BF16, name=f"qt{p}", tag=f"qt{p}")
                  for p in range(NPAIR)]
            kt = [singles.tile([128, L], BF16, name=f"kt{p}", tag=f"kt{p}")
                  for p in range(NPAIR)]
            # V natural layout + ones col, per pair
            vt = [singles.tile([128, JC, 2, DA], BF16, name=f"vt{p}",
                               tag=f"vt{p}") for p in range(NPAIR)]
            # normalized O^T as lhsT for the out-proj; per query chunk
            po = [singles.tile([128, NPAIR, ICW], BF16, name=f"po{i}",
                               tag=f"po{i}") for i in range(NIC)]

            # ---- input DMAs (sync HWDGE ring, FIFO): prefix needs
            # w[K0,Q0,V0] + xt0 first; all chunks contiguous/partition ----
            nc.sync.dma_start(wT_sb[:, 0], w6_h[:, 0])   # K0
            nc.sync.dma_start(xt[0][:], x4_h[:, 0])
            nc.sync.dma_start(ones_sb[:], ones_h[:])
            nc.sync.dma_start(wT_sb[:, 1], w6_h[:, 1])   # Q0
            nc.sync.dma_start(wT_sb[:, 2], w6_h[:, 2])   # V0
            nc.sync.dma_start(xt[1][:], x4_h[:, 1])
            nc.sync.dma_start(xt[2][:], x4_h[:, 2])
            nc.sync.dma_start(xt[3][:], x4_h[:, 3])
            nc.sync.dma_start(onesr_sb[:], onesr_h[:])
            nc.sync.dma_start(wT_sb[:, 3], w6_h[:, 3])
            nc.sync.dma_start(wT_sb[:, 4], w6_h[:, 4])
            nc.sync.dma_start(wT_sb[:, 5], w6_h[:, 5])
            nc.sync.dma_start(wp_sb[:], wp2_h[:])

            # ones columns for V (denominator trick), via cheap DVE copy
            for p in range(NPAIR):
                nc.vector.tensor_copy(
                    vt[p][:, :, :, 64:65],
                    ones_sb[:].rearrange("q (a b) -> q a b", a=JC)[:, :, :, None],
                )

            with tc.tile_pool(name="psS", bufs=2, space="PSUM") as psS_pool, \
                 tc.tile_pool(name="psO", bufs=1, space="PSUM") as psO_pool, \
                 tc.tile_pool(name="scr", bufs=2, space="PSUM") as scr:

                # ---- stage-1 work, split into ~1us steps so interleaved
                # filler never delays the scores->exp chain by much ----
                kq_open = {}

                def kqh(p, lc, which, half):
                    # half 0 opens the accumulation tile; half 1 closes it.
                    # While a tile is open no other scr allocation may be
                    # emitted (ring-order discipline).
                    seg = p * 3 + (1 if which == 0 else 0)
                    key = (p, lc, which)
                    if half == 0:
                        kq_open[key] = scr.tile([128, 512], F32, name="ps",
                                                tag="scr")
                    ps = kq_open[key]
                    for kc in range(half * 4, half * 4 + 4):
                        nc.tensor.matmul(
                            ps[:],
                            wT_sb[:, seg, kc, :],
                            xt[lc][:, kc, :],
                            start=(kc == 0),
                            stop=(kc == KC - 1),
                        )
                    if half == 1:
                        dst = qt[p] if which == 0 else kt[p]
                        nc.vector.tensor_copy(
                            dst[:, lc * 512:(lc + 1) * 512], ps[:]
                        )
                        del kq_open[key]

                def kq_unit(p, lc, which):
                    kqh(p, lc, which, 0)
                    kqh(p, lc, which, 1)

                def vlt(p, lc, lt):
                    seg = p * 3 + 2
                    psv = scr.tile([128, 128], F32, name="psv", tag="scr")
                    for kc in range(KC):
                        nc.tensor.matmul(
                            psv[:],
                            xt[lc][:, kc, lt * 128:(lt + 1) * 128],
                            wT_sb[:, seg, kc, :],
                            start=(kc == 0),
                            stop=(kc == KC - 1),
                        )
                    jc = lc * 4 + lt
                    nc.vector.tensor_copy(
                        vt[p][:, jc, :, 0:64],
                        psv[:].rearrange("q (hh d) -> q hh d", hh=2),
                    )

                def v_unit(p, lc):
                    for lt in range(4):
                        vlt(p, lc, lt)

                def proj_tile(ic, it):
                    ot = out_pool.tile([128, DIM], BF16, tag="outp")
                    for oc in range(2):
                        psP = scr.tile([128, 512], F32, tag="scr")
                        for cc in range(2):
                            nc.tensor.matmul(
                                psP[:],
                                po[ic][:, cc, it * 128:(it + 1) * 128],
                                wp_sb[:, cc, oc * 512:(oc + 1) * 512],
                                start=(cc == 0),
                                stop=(cc == 1),
                            )
                        nc.vector.tensor_copy(
                            ot[:, oc * 512:(oc + 1) * 512], psP[:]
                        )
                    l0 = ic * ICW + it * 128
                    nc.sync.dma_start(out_h[l0:l0 + 128, :], ot[:])

                # ---- prefix: pair-0 lc0 projections ----
                kq_unit(0, 0, 1)   # K0(lc0)
                kq_unit(0, 0, 0)   # Q0(lc0)
                v_unit(0, 0)       # V0(lc0)

                def attention_pair(p, sched=None):
                    sched = sched or {}
                    dyn = {}
                    psO = [None, None]
                    e_tiles = {}

                    def scores_block(ic, jj):
                        # both heads of the pair: concurrent row-tiled MMs
                        psS = psS_pool.tile([128, 2, 512], F32, name="psS",
                                            tag="psS")
                        for hh in range(2):
                            nc.tensor.matmul(
                                psS[:, hh, :],
                                kt[p][hh * 64:(hh + 1) * 64,
                                      jj * 128:(jj + 1) * 128],
                                qt[p][hh * 64:(hh + 1) * 64,
                                      ic * ICW:(ic + 1) * ICW],
                                start=True,
                                stop=True,
                            )
                        e = e_pool.tile([128, 2, 512], BF16, tag="e")
                        nc.scalar.activation(
                            e[:], psS[:],
                            mybir.ActivationFunctionType.Exp,
                            scale=float(SCALE),
                        )
                        e_tiles[(ic, jj)] = e

                    def pv_block(ic, jj):
                        e = e_tiles.pop((ic, jj))
                        for hh in range(2):
                            nc.tensor.matmul(
                                psO[hh][:],
                                vt[p][:, jj, hh, :],
                                e[:, hh, :],
                                start=(jj == 0),
                                stop=(jj == JC - 1),
                            )

                    def normalize_b(ic, d):
                        # broadcast raw denominators across partitions by
                        # DMA (stride-0 source), reciprocal on 128
                        # partitions, scale po. Deferred a block so the
                        # DVE-heavy reciprocal never blocks stage-1 copies
                        # at a chunk boundary.
                        rb = scr.tile([128, 512], F32, name="rb", tag="scr")
                        for hh in range(2):
                            nc.tensor.matmul(
                                rb[hh * 64:(hh + 1) * 64, :],
                                onesr_sb[:],
                                d[0:1, hh * 512:(hh + 1) * 512],
                                start=True,
                                stop=True,
                            )
                        rr = dpool.tile([128, 512], F32, name="rr", tag="r")
                        nc.vector.reciprocal(rr[:], rb[:])
                        nc.vector.tensor_mul(
                            po[ic][:, p, :], po[ic][:, p, :], rr[:]
                        )

                    def normalize_a(ic, bt):
                        # stage denominators + po rows out of PSUM fast,
                        # then free psO for the next chunk
                        d = dpool.tile([1, 1024], F32, name="d", tag="d")
                        for hh in range(2):
                            nc.vector.tensor_copy(
                                d[0:1, hh * 512:(hh + 1) * 512],
                                psO[hh][64:65, :],
                            )
                            nc.vector.tensor_copy(
                                po[ic][hh * 64:(hh + 1) * 64, p, :],
                                psO[hh][0:64, :],
                            )
                        dyn.setdefault(bt + 1, []).append(
                            lambda: normalize_b(ic, d)
                        )
                        if p == 1:
                            for k, it in enumerate(range(4)):
                                dyn.setdefault(bt + 2 + 3 * k, []).append(
                                    lambda ic=ic, it=it: proj_tile(ic, it)
                                )

                    for bt in range(NIC * JC + 1):
                        ic, t = divmod(bt, JC)
                        if bt < NIC * JC:
                            if t == 0:
                                psO[0] = psO_pool.tile(
                                    [DA, 512], F32, name="psOA", tag="psOA")
                                psO[1] = psO_pool.tile(
                                    [DA, 512], F32, name="psOB", tag="psOB")
                            scores_block(ic, t)
                        if bt > 0:
                            pic, pt = divmod(bt - 1, JC)
                            pv_block(pic, pt)
                            if pt == JC - 1:
                                normalize_a(pic, bt)
                        for task in sched.get(bt, ()):
                            task()
                        for task in dyn.pop(bt, ()):
                            task()
                    # drain deferred work emitted past the last block
                    for bt in sorted(dyn):
                        for task in dyn[bt]:
                            task()

                # ---- pair 0 schedule: hard-due steps early (dense, PE
                # saturated during ic0), relaxed singles later, all away
                # from chunk boundaries (16/32/48) ----
                def KQH(p, lc, w, h):
                    return lambda: kqh(p, lc, w, h)

                def VLT(p, lc, lt):
                    return lambda: vlt(p, lc, lt)

                sched0 = {
                    0: [KQH(0, 1, 1, 0)],
                    1: [KQH(0, 1, 1, 1)],
                    2: [VLT(0, 1, 0), VLT(0, 1, 1)],
                    3: [VLT(0, 1, 2), VLT(0, 1, 3)],
                    4: [KQH(0, 2, 1, 0)],
                    5: [KQH(0, 2, 1, 1)],
                    6: [VLT(0, 2, 0), VLT(0, 2, 1)],
                    7: [VLT(0, 2, 2), VLT(0, 2, 3)],
                    8: [KQH(0, 3, 1, 0)],
                    9: [KQH(0, 3, 1, 1)],
                    10: [VLT(0, 3, 0), VLT(0, 3, 1)],
                    11: [VLT(0, 3, 2), VLT(0, 3, 3)],
                    12: [KQH(0, 1, 0, 0)],
                    13: [KQH(0, 1, 0, 1)],
                    18: [KQH(1, 0, 1, 0)],
                    19: [KQH(1, 0, 1, 1)],
                    20: [VLT(1, 0, 0)],
                    21: [VLT(1, 0, 1)],
                    22: [VLT(1, 0, 2)],
                    23: [VLT(1, 0, 3)],
                    24: [KQH(1, 1, 1, 0)],
                    25: [KQH(1, 1, 1, 1)],
                    26: [KQH(0, 2, 0, 0)],
                    27: [KQH(0, 2, 0, 1)],
                    28: [VLT(1, 1, 0)],
                    29: [VLT(1, 1, 1)],
                    34: [VLT(1, 1, 2)],
                    35: [VLT(1, 1, 3)],
                    36: [KQH(1, 2, 1, 0)],
                    37: [KQH(1, 2, 1, 1)],
                    38: [VLT(1, 2, 0)],
                    39: [VLT(1, 2, 1)],
                    40: [KQH(1, 3, 1, 0)],
                    41: [KQH(1, 3, 1, 1)],
                    42: [KQH(0, 3, 0, 0)],
                    43: [KQH(0, 3, 0, 1)],
                    44: [VLT(1, 2, 2)],
                    45: [VLT(1, 2, 3)],
                    50: [VLT(1, 3, 0)],
                    51: [VLT(1, 3, 1)],
                    52: [VLT(1, 3, 2)],
                    53: [VLT(1, 3, 3)],
                    54: [KQH(1, 0, 0, 0)],
                    55: [KQH(1, 0, 0, 1)],
                }
                attention_pair(0, sched=sched0)

                # ---- pair 1: remaining Q projections mid-chunk; proj
                # tiles are scheduled dynamically after each normalize ----
                sched1 = {
                    4: [KQH(1, 1, 0, 0)],
                    5: [KQH(1, 1, 0, 1)],
                    20: [KQH(1, 2, 0, 0)],
                    21: [KQH(1, 2, 0, 1)],
                    36: [KQH(1, 3, 0, 0)],
                    37: [KQH(1, 3, 0, 1)],
                }
                attention_pair(1, sched=sched1)
    return nc


_NC_CACHE = None


def _get_nc():
    global _NC_CACHE
    if _NC_CACHE is None:
        _NC_CACHE = _build_nc()
    return _NC_CACHE


def kernel(x, w_qkv, w_proj, b_proj, _trace=False):
    x = np.asarray(x, dtype=np.float32)
    w_qkv = np.asarray(w_qkv, dtype=np.float32)
    w_proj = np.asarray(w_proj, dtype=np.float32)
    b_proj = np.asarray(b_proj, dtype=np.float32)

    nc = _get_nc()
    in_maps = []
    for b in range(B):
        xT = x[b].T  # [DIM, L]
        # x4[p, lc, kc, l'] = xT[kc*128+p, lc*512+l']
        x4 = np.ascontiguousarray(
            xT.reshape(KC, 128, 4, 512).transpose(1, 2, 0, 3)
        ).astype(ml_dtypes.bfloat16)
        for hg in range(4):
            s = C * hg
            segs = []
            for p in range(NPAIR):
                o = s + p * 128
                segs.append(w_qkv[DIM + o:DIM + o + 128])      # K pair p
                segs.append(w_qkv[o:o + 128])                  # Q pair p
                segs.append(w_qkv[2 * DIM + o:2 * DIM + o + 128])  # V
            w_cat = np.concatenate(segs, 0)  # [768, 1024]
            # w6[p, seg, kc, c] = w_cat[seg*128+c, kc*128+p]
            w6 = np.ascontiguousarray(
                w_cat.reshape(6, 128, KC, 128).transpose(3, 0, 2, 1)
            ).astype(ml_dtypes.bfloat16)
            # wp2[p, cc, o] = w_proj[o, s+cc*128+p]
            wp2 = np.ascontiguousarray(
                w_proj[:, s:s + C].T.reshape(2, 128, DIM).transpose(1, 0, 2)
            ).astype(ml_dtypes.bfloat16)
            in_maps.append({
                "x4": x4,
                "w6": w6,
                "wp2": wp2,
                "ones": np.ones((128, 32), ml_dtypes.bfloat16),
                "onesr": np.ones((1, 64), np.float32),
            })

    res = run_bass_kernel_spmd(nc, in_maps, list(range(NCORES)), trace=_trace)
    parts = [res.results[i]["out"].astype(np.float32) for i in range(NCORES)]
    out = np.stack([
        parts[0] + parts[1] + parts[2] + parts[3],
        parts[4] + parts[5] + parts[6] + parts[7],
    ]).astype(np.float32) + b_proj[None, None, :].astype(np.float32)
    if _trace:
        return out, res
    return out
